# revision 39
# baseline (speedup 1.0000x reference)
"""Trainium2 Bass kernel for nn_EncoderPrecision.

Math: two tiny MLPs map x (B,N,Dx) -> (B,N,Dz); transposed to (B,Dz,N)
vectors d=exp(mlp_d) and u=mlp_o. The outputs are structurally sparse:
  D         = diag(d)                                  (B,Dz,N,N)
  Bmat      = diag(d) + superdiag(u[:, :-1])           (upper bidiagonal)
  precision = Bmat^T Bmat + eps*I                      (tridiagonal)
with closed-form bands:
  precision[i,i]   = d_i^2 + u_{i-1}^2 + eps
  precision[i,i+1] = precision[i+1,i] = d_i * u_i
So instead of an N^3 matmul the kernel computes the MLPs (channels on
partitions, tokens on the free dim; fp32r matmuls = 4x PE throughput at
~1e-4 rounding) and scatters only the bands into the pre-zeroed dense
DRAM outputs. Band rows go out as contiguous runs (8B (d,u) pairs for
Bmat, 12B tridiagonal triples for precision) to halve the DMA descriptor
count, spread across the SP and ACT HWDGE rings. The whole thing is
pipelined chunk-major over 512-token slices (tokens are independent
through the MLP), so the descriptor-bound scatter phase for chunk 0
drains while chunk 1 computes.

Sharding: data-parallel over batch B=8, one batch element per core;
weights replicated. Host gathers by stacking the 8 per-core outputs.

Set MM_DTYPE = "f32" for bit-conservative matmuls (~2x slower end to
end, rel err ~6e-6 instead of ~2.6e-4).
"""

import numpy as np

EPS = 0.001
B, N, Dx, H, Dz = 8, 1024, 32, 256, 8
NCORES = 8
P = 128

# "f32" (exact) or "f32r" (4x faster PE, ~1e-4 matmul rounding)
MM_DTYPE = "f32r"

_WEIGHT_SHAPES = {
    "dW0": (Dx, H), "db0": (H,), "dW1": (H, H), "db1": (H,),
    "dW2": (H, Dz), "db2": (Dz,),
    "oW0": (Dx, H), "ob0": (H,), "oW1": (H, H), "ob1": (H,),
    "oW2": (H, Dz), "ob2": (Dz,),
}

_compiled_nc = None


def _emit(ctx, tc, nc, aps):
    import concourse.mybir as mybir
    from concourse.masks import make_identity

    f32 = mybir.dt.float32
    mmdt = f32 if MM_DTYPE == "f32" else mybir.dt.float32r
    AF = mybir.ActivationFunctionType
    NT = N // P          # 8 token chunks of 128
    NCH = N // 512       # 2 psum free-dim chunks of 512
    KH = H // P          # 2 contraction chunks for H=256

    const = ctx.enter_context(tc.tile_pool(name="const", bufs=1))
    work = ctx.enter_context(tc.tile_pool(name="work", bufs=2))
    bands = ctx.enter_context(tc.tile_pool(name="bands", bufs=1))
    psum = ctx.enter_context(tc.tile_pool(name="psum", bufs=8, space="PSUM"))

    def ps_tile(p_dim, f_dim):
        # one shared tag: 1-bank slots (128x512 f32)
        t = psum.tile([P, 512], f32, tag="ps", name="ps")
        return t[:p_dim, :f_dim]

    def rounded(tile_in, shape, tag):
        # fp32r path: matmul operands must be explicitly rounded; the copy
        # runs on Pool (1-input ~= line-rate) to keep DVE free early on
        if MM_DTYPE == "f32":
            return tile_in
        r = const.tile(shape, mmdt, tag=f"{tag}_r", name=f"{tag}_r")
        nc.gpsimd.tensor_copy(r[:], tile_in[:])
        return r

    ident = const.tile([P, P], f32)
    make_identity(nc, ident[:])

    # --- load x (N, Dx); transpose via PE to xT (Dx on partitions).
    # xT is split per pipeline chunk so chunk 0's L1 only depends on the
    # first transposes. ---
    CHX = 512  # must match the pipeline chunk CH below
    x_sb = const.tile([P, NT, Dx], f32)
    nc.sync.dma_start(x_sb[:], aps["x"].rearrange("(c p) d -> p c d", p=P))
    xT = []
    zpad = const.tile([P, CHX], f32)
    nc.any.memzero(zpad[:])
    for hi in range(N // CHX):
        xh = const.tile([P, CHX], mmdt, tag=f"xT{hi}", name=f"xT{hi}")
        # zero the whole chunk (DVE copy doubles as the f32r rounding);
        # the PE transposes then overwrite rows 0..Dx-1 piece by piece
        nc.vector.tensor_copy(xh[:], zpad[:])
        xT.append(xh)

    # --- weights/biases for both branches, L1's first (earliest PE start);
    # loads spread over both HWDGE rings ---
    w0, w1, w2, b0, b1, b2 = {}, {}, {}, {}, {}, {}
    for br in ("d", "o"):
        w0f = const.tile([P, H], f32, tag=f"w0{br}", name=f"w0f{br}")
        nc.any.memzero(w0f[:])  # K padded to 128
        nc.sync.dma_start(w0f[:Dx, :], aps[f"{br}W0"][:])
        w0[br] = rounded(w0f, [P, H], f"w0{br}")
        b0[br] = const.tile([P, KH], f32, tag=f"b0{br}", name=f"b0{br}")
        nc.scalar.dma_start(b0[br][:], aps[f"{br}b0"].rearrange("(o p) -> p o", p=P))
    for br in ("d", "o"):
        w1f = const.tile([P, KH, H], f32, tag=f"w1{br}", name=f"w1f{br}")
        nc.sync.dma_start(
            w1f[:], aps[f"{br}W1"].rearrange("(ko ki) m -> ki ko m", ki=P))
        w1[br] = rounded(w1f, [P, KH, H], f"w1{br}")
        b1[br] = const.tile([P, KH], f32, tag=f"b1{br}", name=f"b1{br}")
        nc.scalar.dma_start(b1[br][:], aps[f"{br}b1"].rearrange("(o p) -> p o", p=P))
        w2f = const.tile([P, KH, Dz], f32, tag=f"w2{br}", name=f"w2f{br}")
        nc.scalar.dma_start(
            w2f[:], aps[f"{br}W2"].rearrange("(ko ki) m -> ki ko m", ki=P))
        w2[br] = rounded(w2f, [P, KH, Dz], f"w2{br}")
        b2[br] = const.tile([Dz, 1], f32, tag=f"b2{br}", name=f"b2{br}")
        nc.scalar.dma_start(b2[br][:], aps[f"{br}b2"][:, None])
    # bias for the d2 = exp(2y + 2b) trick
    b2d2 = const.tile([Dz, 1], f32)
    nc.vector.tensor_scalar_mul(b2d2[:], b2["d"][:], 2.0)

    # --- chunk-major pipeline: tokens are independent through the whole
    # MLP, so run L1->L2->L3->bands->scatter per 512-token slice. Chunk 0's
    # scatters (the descriptor-bound phase) start while chunk 1 computes.
    CH = 512
    NC3 = N // CH
    d_flat = aps["d_out"].rearrange("z a b -> z (a b)")
    b_flat = aps["b_out"].rearrange("z a b -> z (a b)")
    p_flat = aps["p_out"].rearrange("z a b -> z (a b)")
    NN, S = N * N, N + 1

    h0, h1 = {}, {}
    for br in ("d", "o"):
        h0[br] = work.tile([P, KH, N], mmdt, tag=f"h0{br}", name=f"h0{br}")
        h1[br] = work.tile([P, KH, N], mmdt, tag=f"h1{br}", name=f"h1{br}")
    bpair = bands.tile([Dz, 2 * N], f32)  # even: d = exp(y+b), odd: u = y+b
    d2 = bands.tile([Dz, N], f32)         # d^2 = exp(2y+2b)
    u2 = bands.tile([Dz, N], f32)         # u^2 = (y+b)^2
    ptri = bands.tile([Dz, 3 * (N - 2)], f32)
    crn = bands.tile([Dz, 4], f32)        # P corner pairs

    def relu_store(br, dst, ps, bias):
        # branch d's relus on ACT, branch o's on DVE: the two branches'
        # activation chains run on different engines in parallel
        if br == "d":
            nc.scalar.activation(dst, ps, AF.Relu, bias=bias)
        else:
            nc.vector.tensor_scalar(
                dst, ps, bias, 0.0, mybir.AluOpType.add, mybir.AluOpType.max)

    for c in range(NC3):
        ns = slice(c * CH, (c + 1) * CH)
        # PE transposes for this chunk's x slice (keeps chunk 0's L1 early)
        for t in range(c * CH // P, (c + 1) * CH // P):
            pt = ps_tile(Dx, P)
            nc.tensor.transpose(pt[:], x_sb[:, t, :], ident[:])
            off = (t * P) % CH
            nc.vector.tensor_copy(xT[c][:Dx, off : off + P], pt[:])
        # layer 1: h0T[m] = relu(W0[:, m].T @ xT + b0)
        for br in ("d", "o"):
            for m in range(KH):
                ps = ps_tile(P, CH)
                nc.tensor.matmul(
                    ps[:], w0[br][:, m * P : (m + 1) * P], xT[c][:],
                    start=True, stop=True)
                relu_store(br, h0[br][:, m, ns], ps[:], b0[br][:, m : m + 1])
        # layer 2: h1T[m] = relu(sum_k W1[k, m].T @ h0T[k] + b1)
        for br in ("d", "o"):
            for m in range(KH):
                ps = ps_tile(P, CH)
                for k in range(KH):
                    nc.tensor.matmul(
                        ps[:], w1[br][:, k, m * P : (m + 1) * P],
                        h0[br][:, k, ns],
                        start=(k == 0), stop=(k == KH - 1))
                relu_store(br, h1[br][:, m, ns], ps[:], b1[br][:, m : m + 1])
        # layer 3: yT = sum_k W2[k].T @ h1T[k] (psums consumed by band phase)
        ps3 = {}
        for br in ("d", "o"):
            ps3[br] = ps_tile(Dz, CH)
            for k in range(KH):
                nc.tensor.matmul(
                    ps3[br][:], w2[br][:, k, :], h1[br][:, k, ns],
                    start=(k == 0), stop=(k == KH - 1))

        # band vectors for this chunk (ACT + DVE in parallel)
        nc.scalar.activation(
            bpair[:, 2 * c * CH : 2 * (c + 1) * CH : 2], ps3["d"][:],
            AF.Exp, bias=b2["d"][:, 0:1])
        nc.vector.tensor_scalar_add(
            bpair[:, 2 * c * CH + 1 : 2 * (c + 1) * CH : 2], ps3["o"][:],
            b2["o"][:, 0:1])
        nc.scalar.activation(d2[:, ns], ps3["d"][:], AF.Exp,
                             bias=b2d2[:, 0:1], scale=2.0)
        nc.scalar.activation(u2[:, ns], ps3["o"][:], AF.Square,
                             bias=b2["o"][:, 0:1])

        # tridiagonal rows r (precision row a = r+1): this chunk covers
        # r in [max(0, c*CH-1), min((c+1)*CH-1, N-2))
        r0 = max(0, c * CH - 1)
        r1 = min((c + 1) * CH - 1, N - 2)
        # mid: pdiag[a] = (d2[a] + EPS) + u2[a-1]
        nc.vector.scalar_tensor_tensor(
            ptri[:, 3 * r0 + 1 : 3 * r1 : 3],
            d2[:, r0 + 1 : r1 + 1], EPS, u2[:, r0:r1],
            mybir.AluOpType.add, mybir.AluOpType.add)
        # left: poff[r] = d[r] * u[r]  (Pool)
        nc.gpsimd.tensor_mul(
            ptri[:, 3 * r0 : 3 * r1 : 3],
            bpair[:, 2 * r0 : 2 * r1 : 2], bpair[:, 2 * r0 + 1 : 2 * r1 : 2])
        # right: poff[r+1] = d[r+1] * u[r+1]  (Pool, in parallel with DVE)
        nc.gpsimd.tensor_mul(
            ptri[:, 3 * r0 + 2 : 3 * r1 : 3],
            bpair[:, 2 * r0 + 2 : 2 * r1 + 2 : 2],
            bpair[:, 2 * r0 + 3 : 2 * r1 + 3 : 2])
        if c == 0:
            # P row-0 corner pair (pdiag[0], poff[0])
            nc.vector.tensor_scalar_add(crn[:, 0:1], d2[:, 0:1], EPS)
            nc.vector.tensor_mul(crn[:, 1:2], bpair[:, 0:1], bpair[:, 1:2])
        if c == NC3 - 1:
            # P row-(N-1) corner pair (poff[N-2], pdiag[N-1])
            nc.vector.tensor_mul(
                crn[:, 2:3], bpair[:, 2 * N - 4 : 2 * N - 3],
                bpair[:, 2 * N - 3 : 2 * N - 2])
            nc.vector.scalar_tensor_tensor(
                crn[:, 3:4], d2[:, N - 1 : N], EPS, u2[:, N - 2 : N - 1],
                mybir.AluOpType.add, mybir.AluOpType.add)

        # scatters for this chunk
        # D diagonal (4B runs), alternating HWDGE rings
        ddst = d_flat[:, c * CH * S : min((c * CH + CH - 1) * S + 1, NN) : S]
        dsrc = bpair[:, 2 * c * CH : 2 * (c + 1) * CH : 2]
        (nc.sync if c % 2 == 0 else nc.scalar).dma_start(ddst, dsrc)
        # Bmat (d,u) pairs rows c*CH .. min((c+1)*CH, N-1)-1 -> SP ring
        br0, br1 = c * CH, min((c + 1) * CH, N - 1)
        bdst = b_flat[:, br0 * S : br1 * S].rearrange(
            "z (r cc) -> z r cc", cc=S)[:, :, 0:2]
        nc.sync.dma_start(
            bdst, bpair[:, 2 * br0 : 2 * br1].rearrange("z (r cc) -> z r cc", cc=2))
        # precision triples rows a = r0+1 .. r1 -> ACT ring
        pdst = p_flat[:, (r0 + 1) * S - 1 : (r1 + 1) * S - 1].rearrange(
            "z (r cc) -> z r cc", cc=S)[:, :, 0:3]
        nc.scalar.dma_start(
            pdst, ptri[:, 3 * r0 : 3 * r1].rearrange("z (r cc) -> z r cc", cc=3))
        if c == 0:
            nc.scalar.dma_start(p_flat[:, 0:2], crn[:, 0:2])
        if c == NC3 - 1:
            nc.scalar.dma_start(p_flat[:, NN - 2 : NN], crn[:, 2:4])

    # last B corner: B[N-1,N-1] = d[N-1]
    nc.sync.dma_start(b_flat[:, NN - 1 : NN], bpair[:, 2 * N - 2 : 2 * N - 1])


def _build():
    import concourse.mybir as mybir
    import concourse.tile as tile
    from concourse import bacc
    from contextlib import ExitStack

    f32 = mybir.dt.float32
    nc = bacc.Bacc(
        "TRN2",
        target_bir_lowering=False,
        debug=False,
        enable_asserts=False,
        num_devices=NCORES,
    )
    aps = {"x": nc.dram_tensor("x", (N, Dx), f32, kind="ExternalInput").ap()}
    for name, shape in _WEIGHT_SHAPES.items():
        aps[name] = nc.dram_tensor(name, shape, f32, kind="ExternalInput").ap()
    for name in ("d_out", "b_out", "p_out"):
        aps[name] = nc.dram_tensor(name, (Dz, N, N), f32, kind="ExternalOutput").ap()

    with tile.TileContext(nc) as tc, ExitStack() as ctx:
        _emit(ctx, tc, nc, aps)
    nc.compile()
    return nc


def _get_nc():
    global _compiled_nc
    if _compiled_nc is None:
        _compiled_nc = _build()
    return _compiled_nc


def _run(trace=False, **inputs):
    from concourse.bass_utils import run_bass_kernel_spmd

    nc = _get_nc()
    x = np.ascontiguousarray(np.asarray(inputs["x"], dtype=np.float32))
    weights = {
        k: np.ascontiguousarray(np.asarray(inputs[k], dtype=np.float32))
        for k in _WEIGHT_SHAPES
    }
    in_maps = []
    for i in range(NCORES):
        m = {"x": np.ascontiguousarray(x[i])}
        m.update(weights)
        in_maps.append(m)
    out = run_bass_kernel_spmd(nc, in_maps, core_ids=list(range(NCORES)), trace=trace)
    res = out.results
    D = np.stack([res[i]["d_out"] for i in range(NCORES)])
    Bm = np.stack([res[i]["b_out"] for i in range(NCORES)])
    Pr = np.stack([res[i]["p_out"] for i in range(NCORES)])
    return (D, Bm, Pr), out


def kernel(**inputs):
    outs, _ = _run(trace=False, **inputs)
    return outs


def kernel_profiled(**inputs):
    """Like kernel() but with NTFF tracing; returns (outputs, BassKernelResults).
    Falls back to untraced execution when the axon NTFF hook is unavailable."""
    try:
        return _run(trace=True, **inputs)
    except ModuleNotFoundError:
        return _run(trace=False, **inputs)
